# revision 1
# baseline (speedup 1.0000x reference)
"""GRU actor (B=1024, T=512, D=64, H=256) on 8 TRN2 NeuronCores.

Single-chain (N=128 matmuls) with critical-path surgery:
  - r- and n-gate PSUMs accumulate W@t1 + W@h_prev - W@(zc*h_prev) instead
    of W@h', so only the eight t1-matmuls gate the next step; the h'-add,
    z-gates, x-side matmuls and zc*h product all hide in idle windows.
  - zc = 1-z comes straight from ACT via sigmoid(-z_pre); per-step ACT is
    exactly sigmoid_r -> sigmoid_zc -> tanh, which packs the in-order queue.
  - A ~9us back-to-back matmul burst at kernel start flips the PE HAM
    clock-gate to 8/8 (2.4 GHz); steady-state idle gaps are too short to
    re-throttle, but steady-state bursts alone cannot warm a cold PE.

Layout transposed on-chip ([feature, batch]):
  gates[3H, B] = W_hh @ h[H, B] + W_ih_aug @ x_aug[D+1, B]
r,z x-side biases folded into an augmented ones-row of x; i_n (+b_ih_n)
host-precomputed, streamed via DMA; b_hh_n added via a K=1 ones matmul.
"""

import numpy as np
import ml_dtypes

LAST_RESULTS = None

import concourse.mybir as mybir
from concourse import bass, bacc
from concourse.tile import TileContext
from concourse.bass_utils import run_bass_kernel_spmd

BF = mybir.dt.bfloat16
F32 = mybir.dt.float32
AF = mybir.ActivationFunctionType
ALU = mybir.AluOpType

B, T, D, H = 1024, 512, 64, 256
NCORES = 8
BC = B // NCORES  # 128 batch rows per core
XBLK = 32  # timesteps per DMA block


def build_nc():
    nc = bacc.Bacc()

    xt = nc.declare_dram_parameter("xt", [D + 1, T, BC], BF, isOutput=False)
    wih = nc.declare_dram_parameter("wih", [D + 1, 4, 128], BF, isOutput=False)
    whh = nc.declare_dram_parameter("whh", [128, 2, 6, 128], BF, isOutput=False)
    whhrn = nc.declare_dram_parameter("whhrn", [128, 2, 4, 128], BF, isOutput=False)
    bhn = nc.declare_dram_parameter("bhn", [1, 2, 128], BF, isOutput=False)
    wbase = nc.declare_dram_parameter("wbase", [128, 2, 2, 128], BF, isOutput=False)
    bbase = nc.declare_dram_parameter("bbase", [1, 2, 128], BF, isOutput=False)
    wdir = nc.declare_dram_parameter("wdir", [128, 2, 8], BF, isOutput=False)
    wmag = nc.declare_dram_parameter("wmag", [128, 2, 8], BF, isOutput=False)
    bdm = nc.declare_dram_parameter("bdm", [1, 2, 8], BF, isOutput=False)
    inx = nc.declare_dram_parameter("inx", [128, T, 2, BC], BF, isOutput=False)
    out = nc.declare_dram_parameter("out", [8, BC], F32, isOutput=True)

    with TileContext(nc) as tc:
        with (
            tc.tile_pool(name="const", bufs=1) as cpool,
            tc.tile_pool(name="xpool", bufs=2) as xpool,
            tc.tile_pool(name="state", bufs=2) as spool,
            tc.tile_pool(name="work", bufs=3) as wpool,
            tc.tile_pool(name="psum", bufs=2, space="PSUM") as ppool,
        ):
            wih_sb = cpool.tile([D + 1, 4, 128], BF)
            nc.sync.dma_start(out=wih_sb[:], in_=wih[:])
            whh_sb = cpool.tile([128, 2, 6, 128], BF)
            nc.sync.dma_start(out=whh_sb[:], in_=whh[:])
            whhrn_sb = cpool.tile([128, 2, 4, 128], BF)
            nc.sync.dma_start(out=whhrn_sb[:], in_=whhrn[:])
            bhn_sb = cpool.tile([1, 2, 128], BF)
            nc.sync.dma_start(out=bhn_sb[:], in_=bhn[:])
            wbase_sb = cpool.tile([128, 2, 2, 128], BF)
            nc.sync.dma_start(out=wbase_sb[:], in_=wbase[:])
            bbase_sb = cpool.tile([1, 2, 128], BF)
            nc.sync.dma_start(out=bbase_sb[:], in_=bbase[:])
            wdir_sb = cpool.tile([128, 2, 8], BF)
            nc.sync.dma_start(out=wdir_sb[:], in_=wdir[:])
            wmag_sb = cpool.tile([128, 2, 8], BF)
            nc.sync.dma_start(out=wmag_sb[:], in_=wmag[:])
            bdm_sb = cpool.tile([1, 2, 8], BF)
            nc.sync.dma_start(out=bdm_sb[:], in_=bdm[:])

            ones_sb = cpool.tile([1, BC], BF)
            nc.vector.memset(ones_sb[:], 1.0)

            # PE warmup: >4us of back-to-back matmuls so the HAM clock-gate
            # flips to 8/8 (2.4 GHz) before the recurrence starts. Steady-state
            # idle gaps (<1us) never re-throttle once warm, but steady-state
            # bursts (~2us) are too short to warm a cold PE on their own.
            with tc.tile_pool(name="warm", bufs=1, space="PSUM") as warmpool:
                warm_ps = warmpool.tile([128, 512], F32, tag="warm")
                for _ in range(40):
                    nc.tensor.matmul(
                        warm_ps[:], whh_sb[:, 0, 0], whh_sb[:, 0, 0:4],
                        start=True, stop=True, skip_group_check=True,
                    )

            h = spool.tile([128, 2, BC], BF, tag="h")
            nc.vector.memset(h[:], 0.0)
            prev_w = None
            prev_t1 = None

            for blk in range(T // XBLK):
                xt_sb = xpool.tile([D + 1, XBLK, BC], BF, tag="xt")
                nc.sync.dma_start(
                    out=xt_sb[:], in_=xt[:, blk * XBLK : (blk + 1) * XBLK, :]
                )
                in_sb = xpool.tile([128, XBLK, 2, BC], BF, tag="inx")
                nc.sync.dma_start(
                    out=in_sb[:], in_=inx[:, blk * XBLK : (blk + 1) * XBLK]
                )
                for j in range(XBLK):
                    t = blk * XBLK + j
                    r_ps = ppool.tile([128, 2, BC], F32, tag="r", name="r_ps")
                    z_ps = ppool.tile([128, 2, BC], F32, tag="z", name="z_ps")
                    n_ps = ppool.tile([128, 2, BC], F32, tag="n", name="n_ps")
                    xcol = xt_sb[:, j]

                    # ---- hoistable matmuls (x-side + n bias) ----
                    for g in range(2):
                        nc.tensor.matmul(
                            r_ps[:, g], wih_sb[:, g], xcol,
                            start=(g == 0), stop=False, skip_group_check=True,
                        )
                    for g in range(2):
                        nc.tensor.matmul(
                            z_ps[:, g], wih_sb[:, 2 + g], xcol,
                            start=(g == 0), stop=(t == 0 and g == 1),
                            skip_group_check=True,
                        )
                    for g in range(2):
                        nc.tensor.matmul(
                            n_ps[:, g], bhn_sb[:, g], ones_sb[:],
                            start=(g == 0), stop=(t == 0 and g == 1),
                            skip_group_check=True,
                        )
                    if t == 0:
                        # h0 = 0: r_ps needs only x; close its group
                        nc.tensor.matmul(
                            r_ps[:, 1], wih_sb[:, 1], xcol,
                            start=False, stop=True, skip_group_check=True,
                        )
                    else:
                        # r,n gates decomposed: W@h' = W@t1 + W@h_prev - W@(zc*h_prev)
                        # so only the t1 matmuls (rT, nT) sit on the critical
                        # path; the h_prev/w2 contributions run in idle windows.
                        for kk in range(2):
                            for g in range(2):
                                nc.tensor.matmul(
                                    r_ps[:, g], whh_sb[:, kk, g], prev_h[:, kk],
                                    start=False, stop=False, skip_group_check=True,
                                )
                            for g in range(2):
                                nc.tensor.matmul(
                                    n_ps[:, g], whh_sb[:, kk, 4 + g], prev_h[:, kk],
                                    start=False, stop=False, skip_group_check=True,
                                )
                        for kk in range(2):
                            for g in range(2):
                                nc.tensor.matmul(
                                    r_ps[:, g], whhrn_sb[:, kk, g], prev_w[:, kk],
                                    start=False, stop=False, skip_group_check=True,
                                )
                            for g in range(2):
                                nc.tensor.matmul(
                                    n_ps[:, g], whhrn_sb[:, kk, 2 + g], prev_w[:, kk],
                                    start=False, stop=False, skip_group_check=True,
                                )
                        # critical: t1 contributions close both groups
                        for g in range(2):
                            for kk in range(2):
                                nc.tensor.matmul(
                                    r_ps[:, g], whh_sb[:, kk, g], prev_t1[:, kk],
                                    start=False, stop=(g == 1 and kk == 1),
                                    skip_group_check=True,
                                )
                        for g in range(2):
                            for kk in range(2):
                                nc.tensor.matmul(
                                    n_ps[:, g], whh_sb[:, kk, 4 + g], prev_t1[:, kk],
                                    start=False, stop=(g == 1 and kk == 1),
                                    skip_group_check=True,
                                )
                        # z gates directly from h (off the critical path)
                        for g in range(2):
                            for kk in range(2):
                                nc.tensor.matmul(
                                    z_ps[:, g], whh_sb[:, kk, 2 + g], h[:, kk],
                                    start=False, stop=(g == 1 and kk == 1),
                                    skip_group_check=True,
                                )
                    # ---- elementwise chain ----
                    sig_r = wpool.tile([128, 2, BC], BF, tag="sr", name="sig_r")
                    nc.scalar.activation(sig_r[:], r_ps[:], AF.Sigmoid)
                    zc = wpool.tile([128, 2, BC], BF, tag="zc", name="zc")
                    nc.scalar.activation(zc[:], z_ps[:], AF.Sigmoid, scale=-1.0)
                    rhn = wpool.tile([128, 2, BC], BF, tag="rhn", name="rhn")
                    nc.vector.tensor_mul(rhn[:], sig_r[:], n_ps[:])
                    npre = wpool.tile([128, 2, BC], BF, tag="npre", name="npre")
                    nc.vector.tensor_add(npre[:], rhn[:], in_sb[:, j])
                    # w2 = zc*h on DVE (2x-mode TT, fills the tanh window)
                    w_t = wpool.tile([128, 2, BC], BF, tag="w", name="w_t")
                    nc.vector.tensor_mul(w_t[:], zc[:], h[:])
                    n_sb = wpool.tile([128, 2, BC], BF, tag="n", name="n_sb")
                    nc.scalar.activation(n_sb[:], npre[:], AF.Tanh)
                    t1 = wpool.tile([128, 2, BC], BF, tag="t1", name="t1")
                    nc.vector.tensor_mul(t1[:], zc[:], n_sb[:])
                    # h' = (t1 - zc*h) + h  (= zc*n + (1-zc)*h)
                    hd = wpool.tile([128, 2, BC], BF, tag="hd", name="hd")
                    nc.vector.tensor_sub(hd[:], t1[:], w_t[:])
                    h_new = spool.tile([128, 2, BC], BF, tag="h")
                    nc.vector.tensor_add(h_new[:], hd[:], h[:])
                    prev_h = h
                    h = h_new
                    prev_w = w_t
                    prev_t1 = t1

            # ---- head MLP on h_T ----
            ones = ones_sb[:]
            ps_base = ppool.tile([128, 2, BC], F32, tag="r")
            for mm in range(2):
                for kk in range(2):
                    nc.tensor.matmul(
                        ps_base[:, mm], wbase_sb[:, kk, mm], h[:, kk],
                        start=(kk == 0), stop=False, skip_group_check=True,
                    )
                nc.tensor.matmul(
                    ps_base[:, mm], bbase_sb[:, mm], ones,
                    start=False, stop=(mm == 1), skip_group_check=True,
                )
            base_sb = wpool.tile([128, 2, BC], BF, tag="base")
            nc.scalar.activation(base_sb[:], ps_base[:], AF.Relu)

            ps_dm = ppool.tile([8, 2, BC], F32, tag="z")
            for which, w_sb in ((0, wdir_sb), (1, wmag_sb)):
                for kk in range(2):
                    nc.tensor.matmul(
                        ps_dm[:, which], w_sb[:, kk], base_sb[:, kk],
                        start=(kk == 0), stop=False, skip_group_check=True,
                    )
                nc.tensor.matmul(
                    ps_dm[:, which], bdm_sb[:, which], ones,
                    start=False, stop=True, skip_group_check=True,
                )
            dir_sb = wpool.tile([8, BC], BF, tag="dir")
            nc.scalar.activation(dir_sb[:], ps_dm[:, 0], AF.Tanh)
            mag_sb = wpool.tile([8, BC], BF, tag="mag")
            nc.scalar.activation(mag_sb[:], ps_dm[:, 1], AF.Sigmoid)
            outf = wpool.tile([8, BC], F32, tag="outf")
            nc.vector.tensor_mul(outf[:], dir_sb[:], mag_sb[:])
            nc.sync.dma_start(out=out[:], in_=outf[:])

    nc.compile()
    return nc


def _prep_shared(w_ih, w_hh, b_ih, b_hh, w_base, b_base, w_dir, b_dir, w_mag, b_mag):
    bf = ml_dtypes.bfloat16
    wih_aug = np.zeros((D + 1, 2 * H), np.float32)
    wih_aug[:D] = w_ih[: 2 * H].T
    wih_aug[D] = b_ih[: 2 * H] + b_hh[: 2 * H]
    wih_p = wih_aug.reshape(D + 1, 4, 128).astype(bf)

    whh_p = (
        w_hh.reshape(6, 128, 2, 128).transpose(3, 2, 0, 1).astype(bf)
    )  # [p, kk, g, m] = w_hh[g*128+m, kk*128+p]
    # negated r- and n-gate tiles: [.., 0:2] = -W_hr, [.., 2:4] = -W_hn
    whhrn_p = np.concatenate(
        [-whh_p[:, :, 0:2], -whh_p[:, :, 4:6]], axis=2
    ).copy()
    bhn_p = b_hh[2 * H :].reshape(1, 2, 128).astype(bf)
    wbase_p = w_base.reshape(2, 128, 2, 128).transpose(3, 2, 0, 1).astype(bf)
    bbase_p = b_base.reshape(1, 2, 128).astype(bf)
    wdir_p = w_dir.T.reshape(2, 128, 8).transpose(1, 0, 2).astype(bf)
    wmag_p = w_mag.T.reshape(2, 128, 8).transpose(1, 0, 2).astype(bf)
    bdm_p = np.stack([b_dir, b_mag]).reshape(1, 2, 8).astype(bf)
    return dict(
        wih=wih_p, whh=whh_p, whhrn=whhrn_p, bhn=bhn_p, wbase=wbase_p,
        bbase=bbase_p, wdir=wdir_p, wmag=wmag_p, bdm=bdm_p,
    )


def _prep_inx(x_shard, w_ih, b_ih):
    gi_n = x_shard.reshape(-1, D).astype(np.float32) @ w_ih[2 * H :].T.astype(np.float32)
    gi_n += b_ih[2 * H :]
    return (
        gi_n.reshape(BC, T, 2, 128).transpose(3, 1, 2, 0).astype(ml_dtypes.bfloat16)
    )


def kernel(x_seq, w_ih, w_hh, b_ih, b_hh, w_base, b_base, w_dir, b_dir,
           w_mag, b_mag, _trace=False, _tmpdir=None):
    bf = ml_dtypes.bfloat16
    shared = _prep_shared(
        w_ih, w_hh, b_ih, b_hh, w_base, b_base, w_dir, b_dir, w_mag, b_mag
    )
    ones_row = np.ones((1, T, BC), np.float32)
    in_maps = []
    for i in range(NCORES):
        shard = x_seq[i * BC : (i + 1) * BC]
        xt_i = np.concatenate(
            [shard.transpose(2, 1, 0), ones_row], axis=0
        ).astype(bf)
        m = dict(shared)
        m["xt"] = xt_i
        m["inx"] = _prep_inx(shard, w_ih, b_ih)
        in_maps.append(m)

    nc = build_nc()
    res = run_bass_kernel_spmd(
        nc, in_maps, core_ids=list(range(NCORES)),
        trace=_trace, tmpdir=_tmpdir,
    )
    global LAST_RESULTS
    LAST_RESULTS = res
    out_full = np.empty((B, 8), np.float32)
    for i in range(NCORES):
        out_full[i * BC : (i + 1) * BC] = res.results[i]["out"].T
    return out_full

